# revision 6
# baseline (speedup 1.0000x reference)
"""Dilated local attention (kernel_size=3, dilation=2) on Trainium2, 8-core SPMD.

Problem: q,k,v [B=4, D=256, N=8192] f32, HEAD_DIM=32 (8 heads), out [B, N, D].
Per position n, head h: logits s_i = SCALE * <q[:,n], k[:,n+2i-2]> for i=0..2
(zero-padded at sequence edges), softmax over {s_0,s_1,s_2} plus six implicit
zero logits (the torch Unfold padding slots), out = sum_i p_i * v[:, n+2i-2].

Sharding: B*H = 32 (batch, head) units; core ci takes 4 units = a contiguous
[128 channel, 8192] block of batch ci//2 (channels (ci%2)*128 ... +128).

v3 — fp16 compute pipeline (PE matmuls 1 cyc/col vs 4 for fp32, DVE 2x), one
merged per-slab pipeline with the softmax normalization folded into a second
Exp via PSUM accumulation of -ln(Z):
  casts:  q,k,v fp32 -> fp16 (q on ACT, k,v on DVE), k/v zero-padded edges
  per 512-slab:
    DVE  products P_i = qc * kc_shift_i            (fp16 TT 2x, 2048 superslab)
    PE   L[12,512](PSUM) = cl^T @ P_i  (x3, scale folded, accumulate, no stop)
    ACT  E1 = Exp(L)   bf16  (bf16 range needed: logits up to ~18)
    PE   S[4@32:36] = csum^T @ E1   (same PSUM bank, col strip 1)
    ACT  LnZ = Ln(S + 6)  fp16      (the +6 = six zero-logit Unfold slots)
    PE   L -= broadcast12(LnZ)      (accumulate crselneg^T @ LnZ, stop)
    ACT  E2 = Exp(L)   fp16         = normalized probs, no division anywhere
    PE   Pbc[128,3*512](PSUM) = csel^T @ E2        (broadcast to channels)
    ACT  Pbs = copy(Pbc)  fp16
    DVE  T_i = Pbs_i * vc_shift_i                  (fp16 TT 2x)
    PE   O[128,512](PSUM) += transpose(T_i)        (3-tap accumulate, n-major)
    DMA  out[n0:n0+512, :] <- O                    (PSUM -> DRAM direct)
"""

import numpy as np
import ml_dtypes

import concourse.bass as bass
import concourse.bacc as bacc
import concourse.mybir as mybir
import concourse.tile as tile
from concourse.bass_utils import run_bass_kernel_spmd

B, D, N = 4, 256, 8192
HD = 32
H = D // HD
SCALE = float(HD) ** -0.5
NCORES = 8
P = 128           # SBUF partitions = 4 units * 32 head dims
UN = P // HD      # units per core
NL = 3 * UN       # logit rows (3 taps * 4 units)
f32 = mybir.dt.float32
f16 = mybir.dt.float16
bf16 = mybir.dt.bfloat16
AF = mybir.ActivationFunctionType
SROW = 32         # partition row where S/LnZ live (col strip 1 of the L bank)


def _consts():
    # lhsT for logit reduction: L[3u+i, n] += SCALE * sum_d P_i[u*32+d, n]
    cl = np.zeros((P, 3 * NL), np.float16)
    for p in range(P):
        u = p // HD
        for i in range(3):
            cl[p, i * NL + 3 * u + i] = SCALE
    # lhsT for group sums: S[u, n] = sum_i E1[3u+i, n]
    csum = np.zeros((NL, UN), ml_dtypes.bfloat16)
    for m in range(NL):
        csum[m, m // 3] = 1.0
    # lhsT subtracting lnZ from all 3 tap rows of its unit: L[3u+i] -= LnZ[u]
    crselneg = np.zeros((UN, NL), np.float16)
    for m in range(NL):
        crselneg[m // 3, m] = -1.0
    # lhsT for probability broadcast: Pbc_i[p, n] = E2[3*(p//32)+i, n]
    csel = np.zeros((NL, 3 * P), np.float16)
    for p in range(P):
        u = p // HD
        for i in range(3):
            csel[3 * u + i, i * P + p] = 1.0
    ident = np.eye(P, dtype=np.float16)
    return cl, csum, crselneg, csel, ident


def build_kernel(nc, n=N, ss=2048, cs=512, reps=1, psum_out_dma=True,
                 psO_bufs=3):
    """Emit the per-core program. ss: product superslab width (= input DMA /
    cast chunk); cs: slab width (512 = one PSUM bank of fp32); reps>1 wraps
    everything (incl. input DMA) in an on-device benchmark loop."""
    assert ss % cs == 0 and n % ss == 0 and cs % P == 0
    spc = ss // cs            # slabs per chunk

    q_d = nc.declare_dram_parameter("q", [P, n], f32, isOutput=False)
    k_d = nc.declare_dram_parameter("k", [P, n], f32, isOutput=False)
    v_d = nc.declare_dram_parameter("v", [P, n], f32, isOutput=False)
    cl_d = nc.declare_dram_parameter("cl", [P, 3 * NL], f16, isOutput=False)
    csum_d = nc.declare_dram_parameter("csum", [NL, UN], bf16, isOutput=False)
    crselneg_d = nc.declare_dram_parameter("crselneg", [UN, NL], f16,
                                           isOutput=False)
    csel_d = nc.declare_dram_parameter("csel", [NL, 3 * P], f16, isOutput=False)
    ident_d = nc.declare_dram_parameter("ident", [P, P], f16, isOutput=False)
    out_d = nc.declare_dram_parameter("out", [n, P], f32, isOutput=True)

    with tile.TileContext(nc) as tc:
        with (
            tc.tile_pool(name="const", bufs=1) as const_pool,
            tc.tile_pool(name="big", bufs=1) as big_pool,
            tc.tile_pool(name="pall", bufs=2) as pall_pool,
            tc.tile_pool(name="e1", bufs=3) as e1_pool,
            tc.tile_pool(name="e2", bufs=3) as e2_pool,
            tc.tile_pool(name="lnz", bufs=3) as lnz_pool,
            tc.tile_pool(name="pbs", bufs=2) as pbs_pool,
            tc.tile_pool(name="tt", bufs=2) as tt_pool,
            tc.tile_pool(name="outsb", bufs=3) as o_pool,
            tc.tile_pool(name="psL", bufs=2, space="PSUM") as psL,
            tc.tile_pool(name="psP", bufs=1, space="PSUM") as psP,
            tc.tile_pool(name="psO", bufs=psO_bufs, space="PSUM") as psO,
        ):
            cl_t = const_pool.tile([P, 3 * NL], f16)
            nc.sync.dma_start(out=cl_t[:], in_=cl_d[:])
            csum_t = const_pool.tile([NL, UN], bf16)
            nc.sync.dma_start(out=csum_t[:], in_=csum_d[:])
            crsel_t = const_pool.tile([SROW + UN, NL], f16)
            nc.sync.dma_start(out=crsel_t[SROW : SROW + UN, :], in_=crselneg_d[:])
            csel_t = const_pool.tile([NL, 3 * P], f16)
            nc.sync.dma_start(out=csel_t[:], in_=csel_d[:])
            ident_t = const_pool.tile([P, P], f16)
            nc.sync.dma_start(out=ident_t[:], in_=ident_d[:])
            bias6 = const_pool.tile([SROW + UN, 1], f32)
            nc.gpsimd.memset(bias6[:], 6.0)

            qb = big_pool.tile([P, n], f32)
            kb = big_pool.tile([P, n], f32)
            vb = big_pool.tile([P, n], f32)
            qc = big_pool.tile([P, n], f16)
            kc = big_pool.tile([P, n + 4], f16)
            vc = big_pool.tile([P, n + 4], f16)
            nc.gpsimd.memset(kc[:, 0:2], 0.0)
            nc.gpsimd.memset(kc[:, n + 2 : n + 4], 0.0)
            nc.gpsimd.memset(vc[:, 0:2], 0.0)
            nc.gpsimd.memset(vc[:, n + 2 : n + 4], 0.0)

            def body():
                for c in range(n // ss):
                    sl = slice(c * ss, (c + 1) * ss)
                    nc.sync.dma_start(out=qb[:, sl], in_=q_d[:, sl])
                    nc.sync.dma_start(out=kb[:, sl], in_=k_d[:, sl])
                    nc.sync.dma_start(out=vb[:, sl], in_=v_d[:, sl])
                for c in range(n // ss):
                    n0 = c * ss
                    ln = min(ss + 4, n - n0)  # pad 4 into next chunk (WAW on
                    nc.scalar.copy(qc[:, n0 : n0 + ss], qb[:, n0 : n0 + ss])
                    nc.vector.tensor_copy(           # same engine = harmless)
                        kc[:, 2 + n0 : 2 + n0 + ln], kb[:, n0 : n0 + ln]
                    )
                    nc.vector.tensor_copy(
                        vc[:, 2 + n0 : 2 + n0 + ln], vb[:, n0 : n0 + ln]
                    )
                    prods(n0)
                    for s in range(c * spc, (c + 1) * spc):
                        slab(s)

            def prods(n0):
                Pall = pall_pool.tile([P, 3 * ss], f16, name="Pall")
                for i in range(3):
                    nc.vector.tensor_mul(
                        Pall[:, i * ss : (i + 1) * ss],
                        qc[:, n0 : n0 + ss],
                        kc[:, n0 + 2 * i : n0 + 2 * i + ss],
                    )
                prods.cur = (n0, Pall)

            def slab(s):
                n0 = s * cs
                p0, Pall = prods.cur
                off = n0 - p0
                L = psL.tile([SROW + UN, cs], f32, name="L")
                for i in range(3):
                    nc.tensor.matmul(
                        L[0:NL, :],
                        cl_t[:, i * NL : (i + 1) * NL],
                        Pall[:, i * ss + off : i * ss + off + cs],
                        start=(i == 0),
                        stop=False,
                    )
                E1 = e1_pool.tile([NL, cs], bf16, name="E1")
                nc.scalar.activation(E1[:], L[0:NL, :], AF.Exp)
                nc.tensor.matmul(
                    L[SROW : SROW + UN, :], csum_t[:], E1[:],
                    start=True, stop=True, tile_position=(0, SROW),
                )
                LnZ = lnz_pool.tile([SROW + UN, cs], f16, name="LnZ")
                nc.scalar.activation(
                    LnZ[SROW : SROW + UN, :], L[SROW : SROW + UN, :],
                    AF.Ln, bias=bias6[SROW : SROW + UN, :],
                )
                nc.tensor.matmul(
                    L[0:NL, :], crsel_t[SROW : SROW + UN, :],
                    LnZ[SROW : SROW + UN, :],
                    start=False, stop=True, tile_position=(SROW, 0),
                )
                E2 = e2_pool.tile([NL, cs], f16, name="E2")
                nc.scalar.activation(E2[:], L[0:NL, :], AF.Exp)
                Pb = psP.tile([P, 3 * cs], f32, name="Pb")
                for i in range(3):
                    nc.tensor.matmul(
                        Pb[:, i * cs : (i + 1) * cs],
                        csel_t[:, i * P : (i + 1) * P],
                        E2[:],
                        start=True, stop=True,
                    )
                Pbs = pbs_pool.tile([P, 3 * cs], f16, name="Pbs")
                nc.scalar.copy(Pbs[:], Pb[:])
                T = tt_pool.tile([P, 3 * cs], f16, name="T")
                for i in range(3):
                    nc.vector.tensor_mul(
                        T[:, i * cs : (i + 1) * cs],
                        Pbs[:, i * cs : (i + 1) * cs],
                        vc[:, n0 + 2 * i : n0 + 2 * i + cs],
                    )
                nc.vector.tensor_add(T[:, 0:cs], T[:, 0:cs], T[:, cs : 2 * cs])
                nc.vector.tensor_add(
                    T[:, 0:cs], T[:, 0:cs], T[:, 2 * cs : 3 * cs]
                )
                O = psO.tile([P, cs], f16, name="O")
                for cc in range(cs // P):
                    nc.tensor.matmul(
                        O[:, cc * P : (cc + 1) * P],
                        T[:, cc * P : cc * P + P],
                        ident_t[:],
                        is_transpose=True,
                        start=True,
                        stop=True,
                    )
                osb = o_pool.tile([P, cs], f32, name="osb")
                nc.scalar.copy(osb[:], O[:])
                osrc = osb[:]
                nc.sync.dma_start(
                    out=out_d[n0 : n0 + cs, :].rearrange(
                        "(cc p) col -> p cc col", p=P
                    ),
                    in_=osrc.rearrange("p (cc col) -> p cc col", cc=cs // P),
                )

            if reps == 1:
                body()
            else:
                with tc.For_i(0, reps, 1):
                    body()
    return nc


_cache = {}


def _get_nc():
    if "nc" not in _cache:
        nc = bacc.Bacc(None, target_bir_lowering=False, debug=False)
        build_kernel(nc)
        nc.compile()
        _cache["nc"] = nc
    return _cache["nc"]


def make_in_maps(q, k, v):
    cl, csum, crselneg, csel, ident = _consts()
    in_maps = []
    for ci in range(NCORES):
        b = ci // 2
        c0 = (ci % 2) * P
        in_maps.append(
            {
                "q": np.ascontiguousarray(q[b, c0 : c0 + P, :]),
                "k": np.ascontiguousarray(k[b, c0 : c0 + P, :]),
                "v": np.ascontiguousarray(v[b, c0 : c0 + P, :]),
                "cl": cl,
                "csum": csum,
                "crselneg": crselneg,
                "csel": csel,
                "ident": ident,
            }
        )
    return in_maps


def run_sharded(q, k, v, **spmd_kwargs):
    q = np.ascontiguousarray(np.asarray(q), dtype=np.float32)
    k = np.ascontiguousarray(np.asarray(k), dtype=np.float32)
    v = np.ascontiguousarray(np.asarray(v), dtype=np.float32)
    assert q.shape == (B, D, N)
    nc = _get_nc()
    in_maps = make_in_maps(q, k, v)
    res = run_bass_kernel_spmd(nc, in_maps, list(range(NCORES)), **spmd_kwargs)
    out = np.empty((B, N, D), np.float32)
    for ci, r in enumerate(res.results):
        b = ci // 2
        c0 = (ci % 2) * P
        out[b, :, c0 : c0 + P] = r["out"]
    return out, res


def kernel(q, k, v):
    return run_sharded(q, k, v)[0]


# revision 33
# speedup vs baseline: 1.2363x; 1.2363x over previous
"""Dilated local attention (kernel_size=3, dilation=2) on Trainium2, 8-core SPMD.

Problem: q,k,v [B=4, D=256, N=8192] f32, HEAD_DIM=32 (8 heads), out [B, N, D].
Per position n, head h: logits s_i = SCALE * <q[:,n], k[:,n+2i-2]> for i=0..2
(zero-padded at sequence edges), softmax over {s_0,s_1,s_2} plus six implicit
zero logits (the torch Unfold padding slots), out = sum_i p_i * v[:, n+2i-2].

Sharding: B*H = 32 (batch, head) units; core ci takes 4 units = a contiguous
[128 channel, 8192] block of batch ci//2 (channels (ci%2)*128 ... +128).

v5 — modulo-scheduled (software-pipelined) fp16 pipeline. The v3/v4 slab
chain ran fully serially (~9.5us/slab): each PE op waited an ACT result of
the SAME slab, so no engine ever ran ahead. v5 emits stage j of slab r-j in
round r so every engine's in-order queue only sees work whose inputs were
produced ~j rounds earlier.

Layout: "tall L" [108, 128] — each 512-slab is 4 subchunks of 128 columns
stacked on 32-aligned partition strips (rows 32j+3u+i, j=subchunk, u=unit,
i=tap). ACT transcendentals then run at FD=128 (345ns) instead of FD=512
(940ns; ACT PSUM-src costs ~1.9 cyc/col).

Stages (slab index at round r):
  St0  PE    logits: 12 MMs cl^T @ products -> psL [108,128]   (+casts/
             products of the NEXT input chunk, spread over the round's engines)
  St1  ACT   E1 = Exp(psL)             bf16 (logit range needs bf16)
  St2  PE    S[16,128] = csum4^T @ E1  (per j,u: sum of 3 tap-exps)
  St3  ACT   R = Reciprocal(S + 6)     bf16 (the +6 = zero-logit Unfold slots)
  St4  DMA   R12sb = broadcast R rows to the 12 rows of each strip (3KB)
  St5  POOL  E2 = E1 * R12sb           bf16 = normalized probs, no PE transit
  St6  PE    Pbc: 12 MMs csel4^T @ E2 -> psP [128, 3*512] f32 (channel bcast)
  St7  DVE   T_i = (psP_i * 1) * vc_i  scalar_tensor_tensor: fused PSUM
             evacuation + V-multiply, one op per tap
  St8  POOL  T = T0+T1+T2
  St9  PE    O = transpose(T) 4x is_transpose -> psO f16
  St10 ACT   osb = copy(psO) f32
  St11 DMA   out[n0:n0+512, :] <- osb
"""

import numpy as np
import ml_dtypes

import concourse.bass as bass
import concourse.bacc as bacc
import concourse.mybir as mybir
import concourse.tile as tile
from concourse.bass_utils import run_bass_kernel_spmd

B, D, N = 4, 256, 8192
HD = 32
H = D // HD
SCALE = float(HD) ** -0.5
NCORES = 8
P = 128
UN = P // HD      # units per core (4)
NL = 3 * UN       # logit rows per strip (12)
SJ = 4            # subchunks per slab
LROWS = 32 * (SJ - 1) + NL   # 108
SROWS = SJ * UN   # 16 rows of S/R
f32 = mybir.dt.float32
f16 = mybir.dt.float16
bf16 = mybir.dt.bfloat16
AF = mybir.ActivationFunctionType
MUL = mybir.AluOpType.mult


def _consts():
    # lhsT for logit reduction (shared across strips), tap-major rows:
    # L[4i+u] += SC*P_i[u*32+d]
    cl = np.zeros((P, 3 * NL), np.float16)
    for p in range(P):
        u = p // HD
        for i in range(3):
            cl[p, i * NL + 4 * i + u] = SCALE
    # lhsT for strip sums broadcast to all 12 rows of each strip:
    # S12[32j+4i+u, c] = sum_i' E1[32j+4i'+u, c]
    csumB = np.zeros((LROWS, LROWS), ml_dtypes.bfloat16)
    for j in range(SJ):
        for u in range(UN):
            for i in range(3):
                for i2 in range(3):
                    csumB[32 * j + 4 * i2 + u, 32 * j + 4 * i + u] = 1.0
    # lhsT for probability broadcast: one full-height zero-padded weight per
    # (tap, strip) — multiple row-strip tile_position matmuls back-to-back
    # crash at runtime, so select the strip with zeros instead.
    # Pbc_ij[p, c] = E2[32j + 4i + (p//32), c]
    csel4 = np.zeros((LROWS, 3 * SJ * P), ml_dtypes.bfloat16)
    for j in range(SJ):
        for p in range(P):
            u = p // HD
            for i in range(3):
                csel4[32 * j + 4 * i + u, (i * SJ + j) * P + p] = 1.0
    ident = np.eye(P, dtype=np.float16)
    zrow = np.zeros((1, LROWS), np.float16)
    return cl, csumB, csel4, ident, zrow


def build_kernel(nc, n=N, ss=2048, cs=512, ldc=1024, reps=1,
                 e2_pool=True, adds_pool=2, stage="full", use_stt=True,
                 zero_init=True, maxstage=12, dbg=False):
    """ss: cast/product chunk; cs: slab width; ldc: input DMA chunk;
    adds_pool: how many of the 2 tap-adds run on GpSimd (rest on DVE)."""
    assert ss % cs == 0 and n % ss == 0 and cs == P * SJ and ss % ldc == 0
    spc = ss // cs
    nslab = n // cs
    nchunk = n // ss

    q_d = nc.declare_dram_parameter("q", [P, n], f32, isOutput=False)
    k_d = nc.declare_dram_parameter("k", [P, n], f32, isOutput=False)
    v_d = nc.declare_dram_parameter("v", [P, n], f32, isOutput=False)
    cl_d = nc.declare_dram_parameter("cl", [P, 3 * NL], f16, isOutput=False)
    csum_d = nc.declare_dram_parameter("csumB", [LROWS, LROWS], bf16,
                                       isOutput=False)
    csel_d = nc.declare_dram_parameter("csel4", [LROWS, 3 * SJ * P], bf16,
                                       isOutput=False)
    ident_d = nc.declare_dram_parameter("ident", [P, P], f16, isOutput=False)
    zrow_d = nc.declare_dram_parameter("zrow", [1, LROWS], f16, isOutput=False)
    out_d = nc.declare_dram_parameter("out", [n, P], f32, isOutput=True)
    if dbg:
        dbg_d = {
            "dbg_e1": nc.declare_dram_parameter("dbg_e1", [LROWS, P], bf16, isOutput=True),
            "dbg_s": nc.declare_dram_parameter("dbg_s", [SROWS, P], f32, isOutput=True),
            "dbg_r": nc.declare_dram_parameter("dbg_r", [SROWS, P], bf16, isOutput=True),
            "dbg_r12": nc.declare_dram_parameter("dbg_r12", [LROWS, P], bf16, isOutput=True),
            "dbg_e2": nc.declare_dram_parameter("dbg_e2", [LROWS, P], bf16, isOutput=True),
            "dbg_pb": nc.declare_dram_parameter("dbg_pb", [P, 3 * P * SJ], f16, isOutput=True),
            "dbg_l": nc.declare_dram_parameter("dbg_l", [LROWS, P], f32, isOutput=True),
        }
        DS = 3  # slab to dump

    with tile.TileContext(nc) as tc:
        with (
            tc.tile_pool(name="const", bufs=1) as const_pool,
            tc.tile_pool(name="big", bufs=1) as big_pool,
            tc.tile_pool(name="pall", bufs=2) as pall_pool,
            tc.tile_pool(name="e1", bufs=6) as e1_pool,
            tc.tile_pool(name="lns", bufs=3) as lns_pool,
            tc.tile_pool(name="rr", bufs=3) as r_pool,
            tc.tile_pool(name="e2", bufs=3) as e2_pool_,
            tc.tile_pool(name="tt", bufs=4) as tt_pool,
            tc.tile_pool(name="outsb", bufs=3) as o_pool,
            tc.tile_pool(name="psL", bufs=2, space="PSUM") as psL,
            tc.tile_pool(name="psS12", bufs=2, space="PSUM") as psS12,
            tc.tile_pool(name="psP", bufs=2, space="PSUM") as psP,
            tc.tile_pool(name="psO", bufs=2, space="PSUM") as psO,
        ):
            cl_t = const_pool.tile([P, 3 * NL], f16)
            nc.sync.dma_start(out=cl_t[:], in_=cl_d[:])
            csum_t = const_pool.tile([LROWS, LROWS], bf16)
            nc.sync.dma_start(out=csum_t[:], in_=csum_d[:])
            csel_t = const_pool.tile([LROWS, 3 * SJ * P], bf16)
            nc.sync.dma_start(out=csel_t[:], in_=csel_d[:])
            ident_t = const_pool.tile([P, P], f16)
            nc.sync.dma_start(out=ident_t[:], in_=ident_d[:])
            zrow_t = const_pool.tile([1, LROWS], f16)
            nc.sync.dma_start(out=zrow_t[:], in_=zrow_d[:])
            bias6 = const_pool.tile([LROWS, 1], f32)
            nc.gpsimd.memset(bias6[:], 6.0)

            qb = big_pool.tile([P, n], f32)
            kb = big_pool.tile([P, n], f32)
            vb = big_pool.tile([P, n], f32)
            qc = big_pool.tile([P, n], f16)
            kc = big_pool.tile([P, n + 4], f16)
            vc = big_pool.tile([P, n + 4], f16)
            nc.gpsimd.memset(kc[:, 0:2], 0.0)
            nc.gpsimd.memset(kc[:, n + 2 : n + 4], 0.0)
            nc.gpsimd.memset(vc[:, 0:2], 0.0)
            nc.gpsimd.memset(vc[:, n + 2 : n + 4], 0.0)

            # one-time init: zero the psL banks (junk rows stay 0 forever ->
            # exp(0)=1 on unused rows, never NaN/Inf) and the R12sb junk rows.
            if zero_init:
                for b in range(2):
                    t = psL.tile([LROWS, P], f32, name="L")
                    nc.tensor.matmul(t[:], zrow_t[:], ident_t[0:1, :],
                                     start=True, stop=True)


            state = {}

            def casts(c):  # cast chunk c on three engines (q:ACT k:ACT v:DVE)
                n0 = c * ss
                ln = min(ss + 4, n - n0)
                yield lambda: nc.scalar.copy(qc[:, n0 : n0 + ss],
                                             qb[:, n0 : n0 + ss])
                yield lambda: nc.scalar.copy(kc[:, 2 + n0 : 2 + n0 + ln],
                                             kb[:, n0 : n0 + ln])
                yield lambda: nc.vector.tensor_copy(
                    vc[:, 2 + n0 : 2 + n0 + ln], vb[:, n0 : n0 + ln])

            def prods(c):
                n0 = c * ss
                Pall = pall_pool.tile([P, 3 * ss], f16, name="Pall")
                for i in range(3):
                    nc.vector.tensor_mul(
                        Pall[:, i * ss : (i + 1) * ss],
                        qc[:, n0 : n0 + ss],
                        kc[:, n0 + 2 * i : n0 + 2 * i + ss],
                    )
                state[("pall", c)] = Pall

            def st0_logits(s):
                Pall = state[("pall", s // spc)]
                off = (s % spc) * cs
                L = psL.tile([LROWS, P], f32, name="L")
                for i in range(3):
                    for j in range(SJ):
                        nc.tensor.matmul(
                            L[32 * j : 32 * j + NL, :],
                            cl_t[:, i * NL : (i + 1) * NL],
                            Pall[:, i * ss + off + P * j : i * ss + off + P * (j + 1)],
                            start=(i == 0),
                            stop=(i == 2),
                            tile_position=(0, 32 * j),
                        )
                state[(0, s)] = L

            def st1_exp1(s):
                L = state.pop((0, s))
                E1 = e1_pool.tile([LROWS, P], bf16, name="E1")
                nc.scalar.activation(E1[:], L[:], AF.Exp)
                if dbg and s == DS:
                    tmp = o_pool.tile([LROWS, P], f32, name="dbgl")
                    nc.scalar.copy(tmp[:], L[:])
                    nc.sync.dma_start(out=dbg_d["dbg_l"][:], in_=tmp[:])
                    nc.sync.dma_start(out=dbg_d["dbg_e1"][:], in_=E1[:])
                state[(1, s)] = E1

            def st2_sum(s):
                E1 = state[(1, s)]
                S12 = psS12.tile([LROWS, P], f32, name="S12")
                nc.tensor.matmul(S12[:], csum_t[:], E1[:], start=True, stop=True)
                state[(2, s)] = S12

            def st3_rcp(s):
                S12 = state.pop((2, s))
                LnS = lns_pool.tile([LROWS, P], f32, name="LnS")
                nc.scalar.activation(LnS[:], S12[:], AF.Ln, bias=bias6[:])
                R12 = r_pool.tile([LROWS, P], bf16, name="R12")
                nc.scalar.activation(R12[:], LnS[:], AF.Exp, scale=-1.0)
                state[(3, s)] = R12

            def st4_e2(s):
                R12 = state.pop((3, s))
                E1 = state.pop((1, s))
                E2 = e2_pool_.tile([LROWS, P], bf16, name="E2")
                nc.vector.tensor_mul(E2[:], E1[:], R12[:])
                if dbg and s == DS:
                    nc.sync.dma_start(out=dbg_d["dbg_e2"][:], in_=E2[:])
                state[(5, s)] = E2

            def st6_pbc_tmul(s):
                E2 = state.pop((5, s))
                n0 = s * cs
                T = tt_pool.tile([P, 3 * cs], f16, name="T")
                for i in range(3):
                    Pb = psP.tile([P, cs], f32, name="Pb")
                    for j in range(SJ):
                        nc.tensor.matmul(
                            Pb[:, P * j : P * (j + 1)],
                            csel_t[:, (i * SJ + j) * P : (i * SJ + j + 1) * P],
                            E2[:],
                            start=True,
                            stop=True,
                        )
                    if use_stt:
                        nc.vector.scalar_tensor_tensor(
                            T[:, i * cs : (i + 1) * cs],
                            Pb[:],
                            1.0,
                            vc[:, n0 + 2 * i : n0 + 2 * i + cs],
                            MUL,
                            MUL,
                        )
                    else:
                        nc.vector.tensor_mul(
                            T[:, i * cs : (i + 1) * cs],
                            Pb[:],
                            vc[:, n0 + 2 * i : n0 + 2 * i + cs],
                        )
                if dbg and s == DS:
                    nc.sync.dma_start(out=dbg_d["dbg_pb"][:], in_=T[:])
                state[(7, s)] = T

            def st8_adds(s):
                T = state[(7, s)]
                for a in range(2):
                    eng = nc.gpsimd if a < adds_pool else nc.vector
                    eng.tensor_add(T[:, 0:cs], T[:, 0:cs],
                                   T[:, (a + 1) * cs : (a + 2) * cs])

            def st9_transp(s):
                T = state.pop((7, s))
                O = psO.tile([P, cs], f16, name="O")
                for cc in range(SJ):
                    nc.tensor.matmul(
                        O[:, cc * P : (cc + 1) * P],
                        T[:, cc * P : cc * P + P],
                        ident_t[:],
                        is_transpose=True,
                        start=True,
                        stop=True,
                    )
                state[(9, s)] = O

            def st10_out(s):
                O = state.pop((9, s))
                n0 = s * cs
                osb = o_pool.tile([P, cs], f32, name="osb")
                nc.scalar.copy(osb[:], O[:])
                if stage == "noout":
                    return
                nc.sync.dma_start(
                    out=out_d[n0 : n0 + cs, :].rearrange(
                        "(cc p) col -> p cc col", p=P),
                    in_=osb[:].rearrange("p (cc col) -> p cc col", cc=SJ),
                )

            def body():
                state.clear()
                for c in range(n // ldc):
                    sl = slice(c * ldc, (c + 1) * ldc)
                    nc.sync.dma_start(out=qb[:, sl], in_=q_d[:, sl])
                    nc.sync.dma_start(out=kb[:, sl], in_=k_d[:, sl])
                    nc.sync.dma_start(out=vb[:, sl], in_=v_d[:, sl])
                if stage == "dma":
                    return
                # prologue: chunk 0 casts + products
                for f in casts(0):
                    f()
                prods(0)
                cast_iters = {}
                for r in range(nslab + 10):
                    # next-chunk casts spread across the round's slots
                    c = r // spc + 1
                    if r % spc == 0 and c < nchunk:
                        cast_iters[c] = casts(c)
                        next(cast_iters[c])()
                    elif r % spc in (1, 2) and c < nchunk:
                        it = cast_iters.get(c)
                        if it is not None:
                            try:
                                next(it)()
                            except StopIteration:
                                pass
                    if r % spc == spc - 1 and c < nchunk:
                        prods(c)
                    if r < nslab and maxstage >= 0:
                        st0_logits(r)
                    if 0 <= r - 1 < nslab and maxstage >= 1:
                        st1_exp1(r - 1)
                    if 0 <= r - 2 < nslab and maxstage >= 2:
                        st2_sum(r - 2)
                    if 0 <= r - 3 < nslab and maxstage >= 3:
                        st3_rcp(r - 3)
                    if 0 <= r - 4 < nslab and maxstage >= 4:
                        st4_e2(r - 4)
                    if 0 <= r - 5 < nslab and maxstage >= 6:
                        st6_pbc_tmul(r - 5)
                    if 0 <= r - 6 < nslab and maxstage >= 7:
                        st8_adds(r - 6)
                    if 0 <= r - 7 < nslab and maxstage >= 8:
                        st9_transp(r - 7)
                    if 0 <= r - 8 < nslab and maxstage >= 9:
                        st10_out(r - 8)

            if reps == 1:
                body()
            else:
                with tc.For_i(0, reps, 1):
                    body()
    return nc


_cache = {}


def _get_nc():
    if "nc" not in _cache:
        nc = bacc.Bacc(None, target_bir_lowering=False, debug=False)
        build_kernel(nc)
        nc.compile()
        _cache["nc"] = nc
    return _cache["nc"]


def make_in_maps(q, k, v):
    cl, csumB, csel4, ident, zrow = _consts()
    in_maps = []
    for ci in range(NCORES):
        b = ci // 2
        c0 = (ci % 2) * P
        in_maps.append(
            {
                "q": np.ascontiguousarray(q[b, c0 : c0 + P, :]),
                "k": np.ascontiguousarray(k[b, c0 : c0 + P, :]),
                "v": np.ascontiguousarray(v[b, c0 : c0 + P, :]),
                "cl": cl,
                "csumB": csumB,
                "csel4": csel4,
                "ident": ident,
                "zrow": zrow,
            }
        )
    return in_maps


def run_sharded(q, k, v, **spmd_kwargs):
    q = np.ascontiguousarray(np.asarray(q), dtype=np.float32)
    k = np.ascontiguousarray(np.asarray(k), dtype=np.float32)
    v = np.ascontiguousarray(np.asarray(v), dtype=np.float32)
    assert q.shape == (B, D, N)
    nc = _get_nc()
    in_maps = make_in_maps(q, k, v)
    res = run_bass_kernel_spmd(nc, in_maps, list(range(NCORES)), **spmd_kwargs)
    out = np.empty((B, N, D), np.float32)
    for ci, r in enumerate(res.results):
        b = ci // 2
        c0 = (ci % 2) * P
        out[b, :, c0 : c0 + P] = r["out"]
    return out, res


def kernel(q, k, v):
    return run_sharded(q, k, v)[0]
